# revision 76
# baseline (speedup 1.0000x reference)
"""Trainium2 Bass kernel for nn_BESNumEigen3qubitModel (v3).

Math reduction (exact): dm0/dm1 and their partial transposes are affine in
rho with the identity fixed, so every eigvalsh in the reference reduces to
eigenvalues of 3 Hermitian 8x8 matrices per batch element: rho, pt_a(rho),
pt_c(rho). With w = eig(rho) ascending, S_k0 = sum of k0 smallest, T_k1 =
sum of k1 largest, mu/nu = eig extrema of pt_a/pt_c:
   beta0 = 1/(1-8 w_min), beta1 = 1/(1-8 w_max)
   loss  = (beta0*(S_k0-k0/8)+k0/8 + beta1*(T_k1-k1/8)+k1/8)^2
           + sum over 4 PPT terms (beta*(ext-1/8)+1/8)^2.

Device algorithm (per core: 4096 batch elems -> 128 partitions x 32 tiles,
3 matrix types -> 96 matrices per partition):
  - Matrix data fp16, layout [128, h(2), i(8), j(8), m(96)] with the matrix
    index m LAST (stride 1) so every DVE operand is packed 2-byte ->
    2x (TensorTensor) / 4x (TensorCopy/TensorScalar) DVE throughput.
    The authoritative diagonal is kept in f32 [128, 8(i), 96(m)]
    (eigenvalues accumulate there at full precision).
  - Cyclic complex Jacobi in XOR-pair rounds (pairs (p, p^r), r = 1..7).
    Per round the 4 pairs' rotation params are computed batched over a
    [128, 4, m] layout (exact: a round's 2x2 pivot blocks are mutually
    disjoint); XOR-pair index sets are affine, so they are plain strided
    views. Then per-pair: 10-op fp16 column update, merged Hermitian row
    restore (rows <- conj(cols); the two racy entries are re-zeroed),
    fp16 diag mirror + annihilated-entry zeros.
  - Two independent round streams interleaved 3:2 with each stream's
    next-round params software-pipelined into the other stream's column
    updates: stream A = rho (21 rounds = 2 full + 1 extra sweep),
    stream B = pt_a/pt_c (14 rounds = 2 sweeps, final round params-only
    since only the f32 diag is read afterwards).
  - Round-1 pivot entries are DMA-prefetched into side tiles so the first
    params chain overlaps the bulk matrix DMA.
  - Tail: 2nd-order perturbative diag correction for rho from the residual
    off-diagonal (corr_i = sum_j |a_ij|^2 (d_i-d_j)/((d_i-d_j)^2+delta)),
    Batcher sort of the rho diag, min/max-reduce of PT diags, loss
    assembly - all split into two m-halves for chain overlap.
Accuracy (vs f64 reference, whole batch): max rel err ~5.3e-3.
"""

import numpy as np

D = 8
BATCH = 32768
NCORES = 8
PER_CORE = BATCH // NCORES       # 4096
NTILES = PER_CORE // 128         # 32 tiles per core
NM = 3 * NTILES                  # 96 matrices per partition (type-major)
MRHO = NTILES                    # rho-only count

# elem strides inside the fp16 matrix tile [2(h), 8(i), 8(j), NM(m)]
SM, SJ, SI, SH = 1, NM, 8 * NM, 64 * NM
ASIZE = 2 * 8 * 8 * NM           # 12288
PDELTA = 1e-6                    # perturbative-correction regularizer

_f32 = np.float32


# ---------------------------------------------------------------- host prep --

def _gellmann_basis(d):
    mats = []
    for j in range(d):
        for k in range(j + 1, d):
            m = np.zeros((d, d), np.complex128); m[j, k] = 1; m[k, j] = 1
            mats.append(m)
    for j in range(d):
        for k in range(j + 1, d):
            m = np.zeros((d, d), np.complex128); m[j, k] = -1j; m[k, j] = 1j
            mats.append(m)
    for l in range(1, d):
        m = np.zeros((d, d), np.complex128)
        m[np.arange(l), np.arange(l)] = 1
        m[l, l] = -l
        mats.append(np.sqrt(2.0 / (l * (l + 1))) * m)
    return np.stack(mats)


def _entry_perm(kind):
    p = np.zeros(64, np.int64)
    for i in range(8):
        for j in range(8):
            if kind == 'a':
                i2, j2 = (j & 4) | (i & 3), (i & 4) | (j & 3)
            else:
                i2, j2 = (i & 6) | (j & 1), (j & 6) | (i & 1)
            p[i * 8 + j] = i2 * 8 + j2
    return p


def _build_maps():
    """[64, 384] f32: (vec,1) -> 128 floats (f = h*64 + i*8 + j) of each of
    rho, pt_a(rho), pt_c(rho)."""
    G = _gellmann_basis(D)
    B = np.zeros((64, 128), np.float64)
    for k in range(63):
        B[k, :64] = G[k].real.reshape(-1)
        B[k, 64:] = G[k].imag.reshape(-1)
    B[63, :64] = (np.eye(D) / D).reshape(-1)

    def float_perm(kind):
        e = _entry_perm(kind)
        return np.concatenate([e, 64 + e])

    M3 = np.concatenate([B, B[:, float_perm('a')], B[:, float_perm('c')]], axis=1)
    return M3.astype(_f32)


_M3 = None


def _host_prep(rho_vec):
    global _M3
    if _M3 is None:
        _M3 = _build_maps()
    vec = rho_vec.astype(np.float64)
    vec = vec / np.linalg.norm(vec, axis=-1, keepdims=True)
    vec_aug = np.concatenate(
        [vec.astype(_f32), np.ones((vec.shape[0], 1), _f32)], axis=1)
    flat = vec_aug @ _M3                                   # [B, 384] f32
    arr = flat.reshape(NCORES, NTILES, 128, 3, 128)        # [core,tile,part,type,f]
    ins = []
    diag_f = np.array([i * 8 + i for i in range(8)])
    for c in range(NCORES):
        a = arr[c]
        # fp16 matrices: [part, f, type, tile] -> [128, f*96 + type*32 + tile]
        m16 = np.ascontiguousarray(
            a.transpose(1, 3, 2, 0).reshape(128, 128 * NM)).astype(np.float16)
        # f32 diag: [part, i, type, tile] -> [128, i*96 + m]
        dg = np.ascontiguousarray(
            a[:, :, :, diag_f].transpose(1, 3, 2, 0).reshape(128, 8 * NM)
        ).astype(_f32)
        ins.append({"mats": m16, "diag": dg})
    return ins


# ------------------------------------------------------------ device kernel --

def _xor_pairs(r):
    return [(i, i ^ r) for i in range(8) if i < (i ^ r)]


def _enum_bits(r):
    """Enumeration bit-steps (descending) for pset = {p: bit_bmax(r)(p)=0},
    enumerated in ascending-p order."""
    bmax = 4 if r >= 4 else (2 if r >= 2 else 1)
    return [b for b in (4, 2, 1) if b != bmax]


# Batcher odd-even mergesort network for 8 elements (19 comparators)
_CE8 = [(0, 1), (2, 3), (4, 5), (6, 7), (0, 2), (1, 3), (4, 6), (5, 7),
        (1, 2), (5, 6), (0, 4), (1, 5), (2, 6), (3, 7), (2, 4), (3, 5),
        (1, 2), (3, 4), (5, 6)]

N_FULL = 2      # full sweeps (all 3 matrix types)
N_RHO = 1       # extra rho-only sweeps


def _build_program(k0, k1):
    import concourse.bass as bass
    import concourse.bacc as bacc
    import concourse.mybir as mybir
    from concourse.tile import TileContext
    from contextlib import ExitStack

    f32 = mybir.dt.float32
    f16 = mybir.dt.float16
    ALU = mybir.AluOpType
    ACT = mybir.ActivationFunctionType

    nc = bacc.Bacc("TRN2")
    mats_d = nc.dram_tensor("mats", [128, ASIZE], f16, kind="ExternalInput")
    diag_d = nc.dram_tensor("diag", [128, 8 * NM], f32, kind="ExternalInput")
    out_d = nc.dram_tensor("out", [128, NTILES], f32, kind="ExternalOutput")

    with ExitStack() as ctx:
        tc = ctx.enter_context(TileContext(nc))
        main = ctx.enter_context(tc.tile_pool(name="main", bufs=1))
        pp = ctx.enter_context(tc.tile_pool(name="pp", bufs=2))
        cp = ctx.enter_context(tc.tile_pool(name="cp", bufs=2))

        A = main.tile([128, ASIZE], f16, name="A")
        Dg = main.tile([128, 8 * NM], f32, name="Dg")
        Aap = A[:]
        Dap = Dg[:]
        pdim = list(Aap.ap[0])

        def av(offset, dims):
            return bass.AP(tensor=Aap.tensor, offset=Aap.offset + offset,
                           ap=[pdim] + dims)

        def dv(offset, dims):
            return bass.AP(tensor=Dap.tensor, offset=Dap.offset + offset,
                           ap=[list(Dap.ap[0])] + dims)

        nc.sync.dma_start(out=Dg[:], in_=diag_d[:, :])
        # Prefetch round-1 pivot entries (f = p*8+q and the im half) on the
        # Act engine's DMA queue so the first params chain fully overlaps the
        # bulk matrix DMA below. r=1 pairs (0,1),(2,3),(4,5),(6,7) -> f = 1,
        # 19, 37, 55 (re) and +64 (im), each an 18-step affine f-row set.
        xpre = main.tile([128, 4 * NM], f16, name="xpre")
        ypre = main.tile([128, 4 * NM], f16, name="ypre")
        md = mats_d[:, :]

        def mrows(foff):
            return bass.AP(tensor=md.tensor, offset=md.offset + foff * NM,
                           ap=[list(md.ap[0]), [18 * NM, 4], [1, NM]])

        nc.sync.dma_start(out=xpre[:], in_=mrows(1))
        nc.sync.dma_start(out=ypre[:], in_=mrows(65))
        NCHUNK = 2
        for ch in range(NCHUNK):
            w = ASIZE // NCHUNK
            nc.sync.dma_start(out=av(ch * w, [[1, w]]),
                              in_=mats_d[:, ch * w:(ch + 1) * w])

        eps30 = main.tile([128, 1], f32, name="eps30")
        nc.vector.memset(eps30[:], 1e-30)
        eps35 = main.tile([128, 1], f32, name="eps35")
        nc.vector.memset(eps35[:], 1e-35)

        with nc.allow_low_precision(reason="fp16 Jacobi data by design"):
            _emit_jacobi(nc, bass, mybir, main, pp, cp, av, dv,
                         eps30, eps35, out_d, k0, k1, xpre, ypre)

    nc.finalize()
    return nc


def _emit_jacobi(nc, bass, mybir, main, pp, cp, av, dv, eps30, eps35,
                 out_d, k0, k1, xpre, ypre):
    f32 = mybir.dt.float32
    f16 = mybir.dt.float16
    ALU = mybir.AluOpType
    ACT = mybir.ActivationFunctionType
    TT = nc.vector.tensor_tensor
    GT = nc.gpsimd.tensor_tensor
    STT = nc.vector.scalar_tensor_tensor

    def make_params(r, m0, mp, xy_override=None):
        """Rotation params for round r, matrices m in [m0, m0+mp).
        Returns ((c16, sr16, s2c), [op thunks]) - thunks emit one op each,
        in dependency order, so callers can interleave them with other work.
        xy_override: (Xv, Yv) APs to read the pivot entries from instead of
        the matrix tile (used for the DMA-prefetched first round)."""
        b1, b2 = _enum_bits(r)  # descending

        def merged(dims):
            if dims[0][0] == 2 * dims[1][0]:
                return [[dims[1][0], 4]] + dims[2:]
            return dims

        sgn = lambda b: -1 if (r & b) else 1
        xdims = merged([[b1 * SI + sgn(b1) * b1 * SJ, 2],
                        [b2 * SI + sgn(b2) * b2 * SJ, 2], [1, mp]])
        if xy_override is not None:
            Xv, Yv = xy_override
        else:
            Xv = av(r * SJ + m0, list(xdims))
            Yv = av(SH + r * SJ + m0, list(xdims))
        appv = dv(m0, merged([[b1 * NM, 2], [b2 * NM, 2], [1, mp]]))
        aqqv = dv(r * NM + m0, merged([[sgn(b1) * b1 * NM, 2],
                                       [sgn(b2) * b2 * NM, 2], [1, mp]]))

        def P(tag, dt=f32):
            return pp.tile([128, 4, mp], dt, tag=f"{tag}{mp}g{m0}", name=tag)[:]

        sqx, sqy, n2p, g = P("sqx"), P("sqy"), P("n2p"), P("g")
        gsq, s2, h, ag = P("gsq"), P("s2"), P("h"), P("ag")
        den, T, sg, T2 = P("den"), P("T"), P("sg"), P("T2")
        t2, cden, u, urb2 = P("t2"), P("cden"), P("u"), P("urb2")
        tb = P("tb")
        c16 = P("c16", f16)
        sr16 = P("sr16", f16)
        s2c = pp.tile([128, 2, 4, mp], f16, tag=f"s2c{mp}g{m0}", name="s2c")[:]

        ops = [
            lambda: nc.scalar.activation(sqx, Xv, ACT.Square, scale=2.0),
            lambda: nc.scalar.activation(sqy, Yv, ACT.Square, scale=2.0),
            lambda: TT(g, appv, aqqv, ALU.subtract),
            lambda: nc.scalar.activation(ag, g, ACT.Abs),
            lambda: nc.scalar.sign(sg, g, bias=eps35[:]),
            lambda: nc.scalar.activation(gsq, g, ACT.Square),
            lambda: TT(n2p, sqx, sqy, ALU.add),
            lambda: TT(s2, gsq, n2p, ALU.add),
            lambda: nc.scalar.activation(h, s2, ACT.Sqrt, bias=eps30[:]),
            lambda: GT(den, ag, h, ALU.add),
            lambda: nc.vector.reciprocal(T, den),
            lambda: GT(T2, T, T, ALU.mult),
            lambda: TT(t2, n2p, T2, ALU.mult),
            lambda: nc.scalar.activation(cden, t2, ACT.Sqrt, bias=1.0),
            lambda: nc.vector.reciprocal(c16, cden),
            lambda: GT(u, T, sg, ALU.mult),
            lambda: STT(urb2, u, 2.0, c16, ALU.mult, ALU.mult),
            lambda: TT(sr16, urb2, Xv, ALU.mult),
            lambda: TT(s2c[:, 0], urb2, Yv, ALU.mult),
            lambda: nc.scalar.activation(s2c[:, 1], s2c[:, 0], ACT.Copy,
                                         scale=-1.0),
            lambda: STT(tb, u, 0.5, n2p, ALU.mult, ALU.mult),
            lambda: GT(appv, appv, tb, ALU.add),
            lambda: GT(aqqv, aqqv, tb, ALU.subtract),
        ]
        return (c16, sr16, s2c), ops

    def emit_pair(k, p, q, m0, mu, mp, coeffs, cofs=0):
        """Column update + Hermitian restore for pair (p, q), m in [m0, m0+mu).
        cofs: m-offset of this update range inside the coefficient tiles."""
        c16, sr16, s2c = coeffs
        UD = [[SH, 2], [SI, 8], [1, mu]]
        UDsw = [[-SH, 2], [SI, 8], [1, mu]]
        colp = av(p * SJ + m0, list(UD))
        colq = av(q * SJ + m0, list(UD))
        colp_sw = av(SH + p * SJ + m0, list(UDsw))
        colq_sw = av(SH + q * SJ + m0, list(UDsw))
        cb = bass.AP(tensor=c16.tensor, offset=c16.offset + k * mp + cofs,
                     ap=[list(c16.ap[0]), [0, 2], [0, 8], [1, mu]])
        srb = bass.AP(tensor=sr16.tensor, offset=sr16.offset + k * mp + cofs,
                      ap=[list(sr16.ap[0]), [0, 2], [0, 8], [1, mu]])
        s2b = bass.AP(tensor=s2c.tensor, offset=s2c.offset + k * mp + cofs,
                      ap=[list(s2c.ap[0]), [4 * mp, 2], [0, 8], [1, mu]])

        def CW(tag):
            return cp.tile([128, 2, 8, mu], f16, tag=f"{tag}{mu}g{m0}",
                           name=tag)[:]

        tP, uP, tQ, uQ = CW("tP"), CW("uP"), CW("tQ"), CW("uQ")
        TT(tP, srb, colq, ALU.mult)
        TT(uP, s2b, colq_sw, ALU.mult)
        GT(tQ, srb, colp, ALU.mult)
        TT(uQ, s2b, colp_sw, ALU.mult)
        TT(colp, cb, colp, ALU.mult)
        TT(colp, colp, tP, ALU.add)
        TT(colp, colp, uP, ALU.add)
        TT(colq, cb, colq, ALU.mult)
        TT(colq, colq, tQ, ALU.subtract)
        TT(colq, colq, uQ, ALU.add)

        # Hermitian row restore (merged rows p,q): rows <- conj(cols).
        # The (p,q)/(q,p) entries race within the merged ops but are
        # explicitly zeroed below.
        dROW = [[(q - p) * SI, 2], [SJ, 8], [1, mu]]
        sCOL = [[(q - p) * SJ, 2], [SI, 8], [1, mu]]
        nc.scalar.copy(av(p * SI + m0, list(dROW)), av(p * SJ + m0, list(sCOL)))
        for rw in (p, q):
            nc.vector.tensor_scalar(
                av(SH + rw * SI + m0, [[SJ, 8], [1, mu]]),
                av(SH + rw * SJ + m0, [[SI, 8], [1, mu]]),
                -1.0, None, ALU.mult)

        # diag mirror (fp16 <- f32 Dg) + annihilated-entry zeros
        mdst = av(p * (SI + SJ) + m0, [[(q - p) * (SI + SJ), 2], [1, mu]])
        msrc = dv(p * NM + m0, [[(q - p) * NM, 2], [1, mu]])
        nc.gpsimd.tensor_copy(mdst, msrc)
        nc.scalar.memzero(av(SH + p * (SI + SJ) + m0,
                             [[(q - p) * SI, 2], [(q - p) * SJ, 2], [1, mu]]))
        nc.gpsimd.memset(av(p * SI + q * SJ + m0,
                            [[(q - p) * (SI - SJ), 2], [1, mu]]), 0.0)

    # ---- sweeps: two type-aligned streams, interleaved 3:2 ----
    # Stream A: rho (m 0..31), N_FULL + N_RHO sweeps = 21 rounds (its extra
    # rho-only sweeps overlap the PT stream instead of trailing serially).
    # Stream B: pt_a/pt_c (m 32..95), N_FULL sweeps = 14 rounds; the last
    # round's column updates are dead (only the f32 diag is read afterwards)
    # and are skipped.
    # Each stream's next-round params are software-pipelined into the other
    # stream's pair updates (zip_emit) to hide the params dependency chain.

    def pair_emitters(r, m0, mu, mp, co, cofs=0):
        return [(lambda k=k, p=p, q=q: emit_pair(k, p, q, m0, mu, mp, co, cofs))
                for k, (p, q) in enumerate(_xor_pairs(r))]

    def zip_emit(blocks, thunks):
        per = (len(thunks) + len(blocks) - 1) // len(blocks) if thunks else 0
        for i, b in enumerate(blocks):
            b()
            for t in thunks[i * per:(i + 1) * per]:
                t()

    A_ROUNDS = [r for s in range(N_FULL + N_RHO) for r in range(1, 8)]
    B_ROUNDS = [r for s in range(N_FULL) for r in range(1, 8)]
    AM0, AMP = 0, MRHO          # rho range
    BM0, BMP = MRHO, NM - MRHO  # pt range

    state = {
        "A": {"rounds": A_ROUNDS, "idx": 0, "m0": AM0, "mp": AMP,
              "co": None, "pend": None},
        "B": {"rounds": B_ROUNDS, "idx": 0, "m0": BM0, "mp": BMP,
              "co": None, "pend": None},
    }
    for nm in ("A", "B"):
        st = state[nm]
        xo = bass.AP(tensor=xpre[:].tensor,
                     offset=xpre[:].offset + st["m0"],
                     ap=[list(xpre[:].ap[0]), [NM, 4], [1, st["mp"]]])
        yo = bass.AP(tensor=ypre[:].tensor,
                     offset=ypre[:].offset + st["m0"],
                     ap=[list(ypre[:].ap[0]), [NM, 4], [1, st["mp"]]])
        st["co"], st["pend"] = make_params(st["rounds"][0], st["m0"], st["mp"],
                                           xy_override=(xo, yo))

    def do_slot(x):
        other = "B" if x == "A" else "A"
        st, so = state[x], state[other]
        i = st["idx"]
        r = st["rounds"][i]
        if st["pend"]:                      # own params not yet emitted
            for t in st["pend"]:
                t()
            st["pend"] = None
        last = i == len(st["rounds"]) - 1
        dead = last and x == "B"            # pt final col updates are dead
        if not dead:
            blocks = pair_emitters(r, st["m0"], st["mp"], st["mp"], st["co"])
            zip_emit(blocks, so["pend"] or [])
            so["pend"] = None
        st["idx"] = i + 1
        if not last:
            st["co"], st["pend"] = make_params(st["rounds"][i + 1],
                                               st["m0"], st["mp"])

    # 3:2 interleave: [A B A B A] x 7 covers 21 A-rounds and 14 B-rounds
    for step in range(7):
        for x in ("A", "A", "B", "A", "B"):
            do_slot(x)
    assert state["A"]["idx"] == len(A_ROUNDS)
    assert state["B"]["idx"] == len(B_ROUNDS)

    # ---- pt_a / pt_c diag min/max over i (final after B-stream) ----
    mn = main.tile([128, 2 * NTILES], f32, name="mn")[:]
    mx = main.tile([128, 2 * NTILES], f32, name="mx")[:]
    ptv = dv(NTILES, [[1, 2 * NTILES], [NM, 8]])
    nc.vector.tensor_reduce(mn, ptv, mybir.AxisListType.X, ALU.min)
    nc.vector.tensor_reduce(mx, ptv, mybir.AxisListType.X, ALU.max)

    # ---- tail (perturbative correction, sort, loss assembly) ----
    # Split into two independent m-halves so the two dependency chains
    # overlap; everything here is elementwise per (partition, m).
    acc_full = main.tile([128, NTILES], f32, tag="acc", name="acc")[:]
    HM = MRHO // 2

    def Q(tag, dt=f32):
        return main.tile([128, 8, 8, HM], dt, tag=tag, name=tag)[:]

    halves = []
    for hx, h0 in enumerate((0, HM)):
        SQ, S, dif, dif2 = Q(f"pSQ{hx}"), Q(f"pS{hx}"), Q(f"pdif{hx}"), Q(f"pdif2{hx}")
        corr = main.tile([128, 8, HM], f32, tag=f"pcorr{hx}", name="pcorr")[:]
        halves.append((h0, SQ, S, dif, dif2, corr))

    def emit_perturb(hx):
        h0, SQ, S, dif, dif2, corr = halves[hx]
        M = HM
        nc.scalar.activation(SQ, av(h0, [[SI, 8], [SJ, 8], [1, M]]), ACT.Square)
        nc.scalar.activation(S, av(SH + h0, [[SI, 8], [SJ, 8], [1, M]]),
                             ACT.Square)
        TT(S, S, SQ, ALU.add)
        TT(dif, dv(h0, [[NM, 8], [0, 8], [1, M]]),
           dv(h0, [[0, 8], [NM, 8], [1, M]]), ALU.subtract)    # d_i - d_j
        nc.scalar.activation(dif2, dif, ACT.Square)
        dif2_flat = bass.AP(tensor=dif2.tensor, offset=dif2.offset,
                            ap=[list(dif2.ap[0]), [1, 64 * M]])
        nc.vector.tensor_scalar(dif2_flat, dif2_flat, PDELTA, None, ALU.add)
        nc.vector.reciprocal(dif2, dif2)                       # R
        TT(SQ, S, dif, ALU.mult)                               # W = S*dif
        TT(SQ, SQ, dif2, ALU.mult)
        Wv = bass.AP(tensor=SQ.tensor, offset=SQ.offset,
                     ap=[list(SQ.ap[0]), [8 * M, 8], [1, M], [M, 8]])
        nc.vector.tensor_reduce(corr, Wv, mybir.AxisListType.X, ALU.add)
        TT(dv(h0, [[NM, 8], [1, M]]), dv(h0, [[NM, 8], [1, M]]), corr, ALU.add)

    def emit_sortassemble(hx):
        h0 = halves[hx][0]
        M = HM
        loc = {i: dv(i * NM + h0, [[1, M]]) for i in range(8)}
        tmin = main.tile([128, M], f32, tag=f"tmin{hx}", name="tmin")[:]
        ce = _CE8 if not (k0 == 4 and k1 == 4) else (_CE8[:16] + [(3, 4)])
        for (i, j) in ce:
            di, dj = loc[i], loc[j]
            TT(tmin, di, dj, ALU.min)
            TT(dj, di, dj, ALU.max)
            nc.vector.tensor_copy(di, tmin)

        def L(name):
            return main.tile([128, M], f32, tag=f"{name}{hx}", name=name)[:]

        mu_min = mn[:, h0:h0 + M]
        mu_max = mx[:, h0:h0 + M]
        nu_min = mn[:, NTILES + h0:NTILES + h0 + M]
        nu_max = mx[:, NTILES + h0:NTILES + h0 + M]
        acc = acc_full[:, h0:h0 + M]
        b0, b1 = L("b0"), L("b1")
        nc.vector.tensor_scalar(b0, loc[0], -8.0, 1.0, ALU.mult, ALU.add)
        nc.vector.reciprocal(b0, b0)
        nc.vector.tensor_scalar(b1, loc[7], -8.0, 1.0, ALU.mult, ALU.add)
        nc.vector.reciprocal(b1, b1)

        assert 1 <= k0 <= 8 and 1 <= k1 <= 8
        t1, t2_ = L("t1"), L("t2")
        sA, sB = L("sA"), L("sB")
        if k0 == 4:
            TT(sA, loc[0], loc[1], ALU.add)
            GT(sB, loc[2], loc[3], ALU.add)
            TT(t1, sA, sB, ALU.add)
        else:
            nc.gpsimd.tensor_copy(t1, loc[0])
            for i in range(1, k0):
                TT(t1, t1, loc[i], ALU.add)
        if k1 == 4:
            TT(sA, loc[7], loc[6], ALU.add)
            GT(sB, loc[5], loc[4], ALU.add)
            TT(t2_, sA, sB, ALU.add)
        else:
            nc.gpsimd.tensor_copy(t2_, loc[7])
            for i in range(6, 7 - k1, -1):
                TT(t2_, t2_, loc[i], ALU.add)
        u0, u1 = L("u0"), L("u1")
        STT(u0, t1, -k0 / 8.0, b0, ALU.add, ALU.mult)
        STT(u1, t2_, -k1 / 8.0, b1, ALU.add, ALU.mult)
        TT(u0, u0, u1, ALU.add)
        nc.vector.tensor_scalar(u0, u0, (k0 + k1) / 8.0, None, ALU.add)
        TT(acc, u0, u0, ALU.mult)
        t3s = [L(f"t3{i}") for i in range(4)]
        for n, (beta, ext) in enumerate(
                ((b0, mu_min), (b1, mu_max), (b0, nu_min), (b1, nu_max))):
            t3 = t3s[n]
            STT(t3, ext, -0.125, beta, ALU.add, ALU.mult)
            nc.vector.tensor_scalar(t3, t3, 0.125, None, ALU.add)
            if n % 2 == 0:
                TT(t3, t3, t3, ALU.mult)
            else:
                GT(t3, t3, t3, ALU.mult)
        TT(t3s[0], t3s[0], t3s[1], ALU.add)
        GT(t3s[2], t3s[2], t3s[3], ALU.add)
        TT(t3s[0], t3s[0], t3s[2], ALU.add)
        TT(acc, acc, t3s[0], ALU.add)

    emit_perturb(0)
    emit_perturb(1)
    emit_sortassemble(0)
    nc.sync.dma_start(out=out_d[:, 0:HM], in_=acc_full[:, 0:HM])
    emit_sortassemble(1)
    nc.sync.dma_start(out=out_d[:, HM:MRHO], in_=acc_full[:, HM:MRHO])


_prog_cache = {}


def kernel(rho_vec, rank0, rank1):
    rho_vec = np.asarray(rho_vec, dtype=np.float32)
    k0 = D - int(rank0)
    k1 = D - int(rank1)
    in_maps = _host_prep(rho_vec)

    from concourse.bass_utils import run_bass_kernel_spmd
    key = (k0, k1)
    if key not in _prog_cache:
        _prog_cache[key] = _build_program(k0, k1)
    nc = _prog_cache[key]
    res = run_bass_kernel_spmd(nc, in_maps, core_ids=list(range(NCORES)))
    return np.concatenate(
        [np.asarray(res.results[c]["out"]).T.reshape(-1) for c in range(NCORES)]
    ).astype(np.float32)


# revision 77
# speedup vs baseline: 1.0024x; 1.0024x over previous
"""Trainium2 Bass kernel for nn_BESNumEigen3qubitModel (v3).

Math reduction (exact): dm0/dm1 and their partial transposes are affine in
rho with the identity fixed, so every eigvalsh in the reference reduces to
eigenvalues of 3 Hermitian 8x8 matrices per batch element: rho, pt_a(rho),
pt_c(rho). With w = eig(rho) ascending, S_k0 = sum of k0 smallest, T_k1 =
sum of k1 largest, mu/nu = eig extrema of pt_a/pt_c:
   beta0 = 1/(1-8 w_min), beta1 = 1/(1-8 w_max)
   loss  = (beta0*(S_k0-k0/8)+k0/8 + beta1*(T_k1-k1/8)+k1/8)^2
           + sum over 4 PPT terms (beta*(ext-1/8)+1/8)^2.

Device algorithm (per core: 4096 batch elems -> 128 partitions x 32 tiles,
3 matrix types -> 96 matrices per partition):
  - Matrix data fp16, layout [128, h(2), i(8), j(8), m(96)] with the matrix
    index m LAST (stride 1) so every DVE operand is packed 2-byte ->
    2x (TensorTensor) / 4x (TensorCopy/TensorScalar) DVE throughput.
    The authoritative diagonal is kept in f32 [128, 8(i), 96(m)]
    (eigenvalues accumulate there at full precision).
  - Cyclic complex Jacobi in XOR-pair rounds (pairs (p, p^r), r = 1..7).
    Per round the 4 pairs' rotation params are computed batched over a
    [128, 4, m] layout (exact: a round's 2x2 pivot blocks are mutually
    disjoint); XOR-pair index sets are affine, so they are plain strided
    views. Then per-pair: 10-op fp16 column update, merged Hermitian row
    restore (rows <- conj(cols); the two racy entries are re-zeroed),
    fp16 diag mirror + annihilated-entry zeros.
  - Two independent round streams interleaved 3:2 with each stream's
    next-round params software-pipelined into the other stream's column
    updates: stream A = rho (21 rounds = 2 full + 1 extra sweep),
    stream B = pt_a/pt_c (14 rounds = 2 sweeps, final round params-only
    since only the f32 diag is read afterwards).
  - Round-1 pivot entries are DMA-prefetched into side tiles so the first
    params chain overlaps the bulk matrix DMA.
  - Tail: 2nd-order perturbative diag correction for rho from the residual
    off-diagonal (corr_i = sum_j |a_ij|^2 (d_i-d_j)/((d_i-d_j)^2+delta)),
    Batcher sort of the rho diag, min/max-reduce of PT diags, loss
    assembly - all split into two m-halves for chain overlap.
Accuracy (vs f64 reference, whole batch): max rel err ~5.3e-3.
"""

import numpy as np

D = 8
BATCH = 32768
NCORES = 8
PER_CORE = BATCH // NCORES       # 4096
NTILES = PER_CORE // 128         # 32 tiles per core
NM = 3 * NTILES                  # 96 matrices per partition (type-major)
MRHO = NTILES                    # rho-only count

# elem strides inside the fp16 matrix tile [2(h), 8(i), 8(j), NM(m)]
SM, SJ, SI, SH = 1, NM, 8 * NM, 64 * NM
ASIZE = 2 * 8 * 8 * NM           # 12288
PDELTA = 1e-6                    # perturbative-correction regularizer

_f32 = np.float32


# ---------------------------------------------------------------- host prep --

def _gellmann_basis(d):
    mats = []
    for j in range(d):
        for k in range(j + 1, d):
            m = np.zeros((d, d), np.complex128); m[j, k] = 1; m[k, j] = 1
            mats.append(m)
    for j in range(d):
        for k in range(j + 1, d):
            m = np.zeros((d, d), np.complex128); m[j, k] = -1j; m[k, j] = 1j
            mats.append(m)
    for l in range(1, d):
        m = np.zeros((d, d), np.complex128)
        m[np.arange(l), np.arange(l)] = 1
        m[l, l] = -l
        mats.append(np.sqrt(2.0 / (l * (l + 1))) * m)
    return np.stack(mats)


def _entry_perm(kind):
    p = np.zeros(64, np.int64)
    for i in range(8):
        for j in range(8):
            if kind == 'a':
                i2, j2 = (j & 4) | (i & 3), (i & 4) | (j & 3)
            else:
                i2, j2 = (i & 6) | (j & 1), (j & 6) | (i & 1)
            p[i * 8 + j] = i2 * 8 + j2
    return p


def _build_maps():
    """[64, 384] f32: (vec,1) -> 128 floats (f = h*64 + i*8 + j) of each of
    rho, pt_a(rho), pt_c(rho)."""
    G = _gellmann_basis(D)
    B = np.zeros((64, 128), np.float64)
    for k in range(63):
        B[k, :64] = G[k].real.reshape(-1)
        B[k, 64:] = G[k].imag.reshape(-1)
    B[63, :64] = (np.eye(D) / D).reshape(-1)

    def float_perm(kind):
        e = _entry_perm(kind)
        return np.concatenate([e, 64 + e])

    M3 = np.concatenate([B, B[:, float_perm('a')], B[:, float_perm('c')]], axis=1)
    return M3.astype(_f32)


_M3 = None


def _host_prep(rho_vec):
    global _M3
    if _M3 is None:
        _M3 = _build_maps()
    vec = rho_vec.astype(np.float64)
    vec = vec / np.linalg.norm(vec, axis=-1, keepdims=True)
    vec_aug = np.concatenate(
        [vec.astype(_f32), np.ones((vec.shape[0], 1), _f32)], axis=1)
    flat = vec_aug @ _M3                                   # [B, 384] f32
    arr = flat.reshape(NCORES, NTILES, 128, 3, 128)        # [core,tile,part,type,f]
    ins = []
    diag_f = np.array([i * 8 + i for i in range(8)])
    for c in range(NCORES):
        a = arr[c]
        # fp16 matrices: [part, f, type, tile] -> [128, f*96 + type*32 + tile]
        m16 = np.ascontiguousarray(
            a.transpose(1, 3, 2, 0).reshape(128, 128 * NM)).astype(np.float16)
        # f32 diag: [part, i, type, tile] -> [128, i*96 + m]
        dg = np.ascontiguousarray(
            a[:, :, :, diag_f].transpose(1, 3, 2, 0).reshape(128, 8 * NM)
        ).astype(_f32)
        ins.append({"mats": m16, "diag": dg})
    return ins


# ------------------------------------------------------------ device kernel --

def _xor_pairs(r):
    return [(i, i ^ r) for i in range(8) if i < (i ^ r)]


def _enum_bits(r):
    """Enumeration bit-steps (descending) for pset = {p: bit_bmax(r)(p)=0},
    enumerated in ascending-p order."""
    bmax = 4 if r >= 4 else (2 if r >= 2 else 1)
    return [b for b in (4, 2, 1) if b != bmax]


# Batcher odd-even mergesort network for 8 elements (19 comparators)
_CE8 = [(0, 1), (2, 3), (4, 5), (6, 7), (0, 2), (1, 3), (4, 6), (5, 7),
        (1, 2), (5, 6), (0, 4), (1, 5), (2, 6), (3, 7), (2, 4), (3, 5),
        (1, 2), (3, 4), (5, 6)]

N_FULL = 2      # full sweeps (all 3 matrix types)
N_RHO = 1       # extra rho-only sweeps


def _build_program(k0, k1):
    import concourse.bass as bass
    import concourse.bacc as bacc
    import concourse.mybir as mybir
    from concourse.tile import TileContext
    from contextlib import ExitStack

    f32 = mybir.dt.float32
    f16 = mybir.dt.float16
    ALU = mybir.AluOpType
    ACT = mybir.ActivationFunctionType

    nc = bacc.Bacc("TRN2")
    mats_d = nc.dram_tensor("mats", [128, ASIZE], f16, kind="ExternalInput")
    diag_d = nc.dram_tensor("diag", [128, 8 * NM], f32, kind="ExternalInput")
    out_d = nc.dram_tensor("out", [128, NTILES], f32, kind="ExternalOutput")

    with ExitStack() as ctx:
        tc = ctx.enter_context(TileContext(nc))
        main = ctx.enter_context(tc.tile_pool(name="main", bufs=1))
        pp = ctx.enter_context(tc.tile_pool(name="pp", bufs=2))
        cp = ctx.enter_context(tc.tile_pool(name="cp", bufs=2))

        A = main.tile([128, ASIZE], f16, name="A")
        Dg = main.tile([128, 8 * NM], f32, name="Dg")
        Aap = A[:]
        Dap = Dg[:]
        pdim = list(Aap.ap[0])

        def av(offset, dims):
            return bass.AP(tensor=Aap.tensor, offset=Aap.offset + offset,
                           ap=[pdim] + dims)

        def dv(offset, dims):
            return bass.AP(tensor=Dap.tensor, offset=Dap.offset + offset,
                           ap=[list(Dap.ap[0])] + dims)

        nc.sync.dma_start(out=Dg[:], in_=diag_d[:, :])
        # Prefetch round-1 pivot entries (f = p*8+q and the im half) on the
        # Act engine's DMA queue so the first params chain fully overlaps the
        # bulk matrix DMA below. r=1 pairs (0,1),(2,3),(4,5),(6,7) -> f = 1,
        # 19, 37, 55 (re) and +64 (im), each an 18-step affine f-row set.
        xpre = main.tile([128, 4 * NM], f16, name="xpre")
        ypre = main.tile([128, 4 * NM], f16, name="ypre")
        md = mats_d[:, :]

        def mrows(foff):
            return bass.AP(tensor=md.tensor, offset=md.offset + foff * NM,
                           ap=[list(md.ap[0]), [18 * NM, 4], [1, NM]])

        nc.sync.dma_start(out=xpre[:], in_=mrows(1))
        nc.sync.dma_start(out=ypre[:], in_=mrows(65))
        NCHUNK = 2
        for ch in range(NCHUNK):
            w = ASIZE // NCHUNK
            nc.sync.dma_start(out=av(ch * w, [[1, w]]),
                              in_=mats_d[:, ch * w:(ch + 1) * w])

        eps30 = main.tile([128, 1], f32, name="eps30")
        nc.vector.memset(eps30[:], 1e-30)
        eps35 = main.tile([128, 1], f32, name="eps35")
        nc.vector.memset(eps35[:], 1e-35)

        with nc.allow_low_precision(reason="fp16 Jacobi data by design"):
            _emit_jacobi(nc, bass, mybir, main, pp, cp, av, dv,
                         eps30, eps35, out_d, k0, k1, xpre, ypre)

    nc.finalize()
    return nc


def _emit_jacobi(nc, bass, mybir, main, pp, cp, av, dv, eps30, eps35,
                 out_d, k0, k1, xpre, ypre):
    f32 = mybir.dt.float32
    f16 = mybir.dt.float16
    ALU = mybir.AluOpType
    ACT = mybir.ActivationFunctionType
    TT = nc.vector.tensor_tensor
    GT = nc.gpsimd.tensor_tensor
    STT = nc.vector.scalar_tensor_tensor

    def make_params(r, m0, mp, xy_override=None):
        """Rotation params for round r, matrices m in [m0, m0+mp).
        Returns ((c16, sr16, s2c), [op thunks]) - thunks emit one op each,
        in dependency order, so callers can interleave them with other work.
        xy_override: (Xv, Yv) APs to read the pivot entries from instead of
        the matrix tile (used for the DMA-prefetched first round)."""
        b1, b2 = _enum_bits(r)  # descending

        def merged(dims):
            if dims[0][0] == 2 * dims[1][0]:
                return [[dims[1][0], 4]] + dims[2:]
            return dims

        sgn = lambda b: -1 if (r & b) else 1
        xdims = merged([[b1 * SI + sgn(b1) * b1 * SJ, 2],
                        [b2 * SI + sgn(b2) * b2 * SJ, 2], [1, mp]])
        if xy_override is not None:
            Xv, Yv = xy_override
        else:
            Xv = av(r * SJ + m0, list(xdims))
            Yv = av(SH + r * SJ + m0, list(xdims))
        appv = dv(m0, merged([[b1 * NM, 2], [b2 * NM, 2], [1, mp]]))
        aqqv = dv(r * NM + m0, merged([[sgn(b1) * b1 * NM, 2],
                                       [sgn(b2) * b2 * NM, 2], [1, mp]]))

        def P(tag, dt=f32):
            return pp.tile([128, 4, mp], dt, tag=f"{tag}{mp}g{m0}", name=tag)[:]

        sqx, sqy, n2p, g = P("sqx"), P("sqy"), P("n2p"), P("g")
        gsq, s2, h, ag = P("gsq"), P("s2"), P("h"), P("ag")
        den, T, sg, T2 = P("den"), P("T"), P("sg"), P("T2")
        t2, cden, u, urb2 = P("t2"), P("cden"), P("u"), P("urb2")
        tb = P("tb")
        c16 = P("c16", f16)
        sr16 = P("sr16", f16)
        s2c = pp.tile([128, 2, 4, mp], f16, tag=f"s2c{mp}g{m0}", name="s2c")[:]

        ops = [
            lambda: nc.scalar.activation(sqx, Xv, ACT.Square, scale=2.0),
            lambda: nc.scalar.activation(sqy, Yv, ACT.Square, scale=2.0),
            lambda: TT(g, appv, aqqv, ALU.subtract),
            lambda: nc.scalar.activation(ag, g, ACT.Abs),
            lambda: nc.scalar.sign(sg, g, bias=eps35[:]),
            lambda: nc.scalar.activation(gsq, g, ACT.Square),
            lambda: TT(n2p, sqx, sqy, ALU.add),
            lambda: TT(s2, gsq, n2p, ALU.add),
            lambda: nc.scalar.activation(h, s2, ACT.Sqrt, bias=eps30[:]),
            lambda: GT(den, ag, h, ALU.add),
            lambda: nc.vector.reciprocal(T, den),
            lambda: GT(T2, T, T, ALU.mult),
            lambda: TT(t2, n2p, T2, ALU.mult),
            lambda: nc.scalar.activation(cden, t2, ACT.Sqrt, bias=1.0),
            lambda: nc.vector.reciprocal(c16, cden),
            lambda: GT(u, T, sg, ALU.mult),
            lambda: STT(urb2, u, 2.0, c16, ALU.mult, ALU.mult),
            lambda: TT(sr16, urb2, Xv, ALU.mult),
            lambda: TT(s2c[:, 0], urb2, Yv, ALU.mult),
            lambda: nc.scalar.activation(s2c[:, 1], s2c[:, 0], ACT.Copy,
                                         scale=-1.0),
            lambda: STT(tb, u, 0.5, n2p, ALU.mult, ALU.mult),
            lambda: GT(appv, appv, tb, ALU.add),
            lambda: GT(aqqv, aqqv, tb, ALU.subtract),
        ]
        return (c16, sr16, s2c), ops

    def emit_pair(k, p, q, m0, mu, mp, coeffs, cofs=0):
        """Column update + Hermitian restore for pair (p, q), m in [m0, m0+mu).
        cofs: m-offset of this update range inside the coefficient tiles."""
        c16, sr16, s2c = coeffs
        UD = [[SH, 2], [SI, 8], [1, mu]]
        UDsw = [[-SH, 2], [SI, 8], [1, mu]]
        colp = av(p * SJ + m0, list(UD))
        colq = av(q * SJ + m0, list(UD))
        colp_sw = av(SH + p * SJ + m0, list(UDsw))
        colq_sw = av(SH + q * SJ + m0, list(UDsw))
        cb = bass.AP(tensor=c16.tensor, offset=c16.offset + k * mp + cofs,
                     ap=[list(c16.ap[0]), [0, 2], [0, 8], [1, mu]])
        srb = bass.AP(tensor=sr16.tensor, offset=sr16.offset + k * mp + cofs,
                      ap=[list(sr16.ap[0]), [0, 2], [0, 8], [1, mu]])
        s2b = bass.AP(tensor=s2c.tensor, offset=s2c.offset + k * mp + cofs,
                      ap=[list(s2c.ap[0]), [4 * mp, 2], [0, 8], [1, mu]])

        def CW(tag):
            return cp.tile([128, 2, 8, mu], f16, tag=f"{tag}{mu}g{m0}",
                           name=tag)[:]

        tP, uP, tQ, uQ = CW("tP"), CW("uP"), CW("tQ"), CW("uQ")
        TT(tP, srb, colq, ALU.mult)
        TT(uP, s2b, colq_sw, ALU.mult)
        GT(tQ, srb, colp, ALU.mult)
        TT(uQ, s2b, colp_sw, ALU.mult)
        TT(colp, cb, colp, ALU.mult)
        TT(colp, colp, tP, ALU.add)
        TT(colp, colp, uP, ALU.add)
        TT(colq, cb, colq, ALU.mult)
        TT(colq, colq, tQ, ALU.subtract)
        TT(colq, colq, uQ, ALU.add)

        # Hermitian row restore (merged rows p,q): rows <- conj(cols).
        # The (p,q)/(q,p) entries race within the merged ops but are
        # explicitly zeroed below.
        dROW = [[(q - p) * SI, 2], [SJ, 8], [1, mu]]
        sCOL = [[(q - p) * SJ, 2], [SI, 8], [1, mu]]
        nc.scalar.copy(av(p * SI + m0, list(dROW)), av(p * SJ + m0, list(sCOL)))
        for rw in (p, q):
            nc.vector.tensor_scalar(
                av(SH + rw * SI + m0, [[SJ, 8], [1, mu]]),
                av(SH + rw * SJ + m0, [[SI, 8], [1, mu]]),
                -1.0, None, ALU.mult)

        # diag mirror (fp16 <- f32 Dg) + annihilated-entry zeros
        mdst = av(p * (SI + SJ) + m0, [[(q - p) * (SI + SJ), 2], [1, mu]])
        msrc = dv(p * NM + m0, [[(q - p) * NM, 2], [1, mu]])
        nc.gpsimd.tensor_copy(mdst, msrc)
        nc.scalar.memzero(av(SH + p * (SI + SJ) + m0,
                             [[(q - p) * SI, 2], [(q - p) * SJ, 2], [1, mu]]))
        nc.gpsimd.memset(av(p * SI + q * SJ + m0,
                            [[(q - p) * (SI - SJ), 2], [1, mu]]), 0.0)

    # ---- sweeps: two type-aligned streams, interleaved 3:2 ----
    # Stream A: rho (m 0..31), N_FULL + N_RHO sweeps = 21 rounds (its extra
    # rho-only sweeps overlap the PT stream instead of trailing serially).
    # Stream B: pt_a/pt_c (m 32..95), N_FULL sweeps = 14 rounds; the last
    # round's column updates are dead (only the f32 diag is read afterwards)
    # and are skipped.
    # Each stream's next-round params are software-pipelined into the other
    # stream's pair updates (zip_emit) to hide the params dependency chain.

    def pair_emitters(r, m0, mu, mp, co, cofs=0):
        return [(lambda k=k, p=p, q=q: emit_pair(k, p, q, m0, mu, mp, co, cofs))
                for k, (p, q) in enumerate(_xor_pairs(r))]

    def zip_emit(blocks, thunks):
        per = len(thunks) // len(blocks) if thunks else 0
        for i, b in enumerate(blocks):
            b()
            hi = (i + 1) * per if i < len(blocks) - 1 else len(thunks)
            for t in thunks[i * per:hi]:
                t()

    A_ROUNDS = [r for s in range(N_FULL + N_RHO) for r in range(1, 8)]
    B_ROUNDS = [r for s in range(N_FULL) for r in range(1, 8)]
    AM0, AMP = 0, MRHO          # rho range
    BM0, BMP = MRHO, NM - MRHO  # pt range

    state = {
        "A": {"rounds": A_ROUNDS, "idx": 0, "m0": AM0, "mp": AMP,
              "co": None, "pend": None},
        "B": {"rounds": B_ROUNDS, "idx": 0, "m0": BM0, "mp": BMP,
              "co": None, "pend": None},
    }
    for nm in ("A", "B"):
        st = state[nm]
        xo = bass.AP(tensor=xpre[:].tensor,
                     offset=xpre[:].offset + st["m0"],
                     ap=[list(xpre[:].ap[0]), [NM, 4], [1, st["mp"]]])
        yo = bass.AP(tensor=ypre[:].tensor,
                     offset=ypre[:].offset + st["m0"],
                     ap=[list(ypre[:].ap[0]), [NM, 4], [1, st["mp"]]])
        st["co"], st["pend"] = make_params(st["rounds"][0], st["m0"], st["mp"],
                                           xy_override=(xo, yo))

    def do_slot(x):
        other = "B" if x == "A" else "A"
        st, so = state[x], state[other]
        i = st["idx"]
        r = st["rounds"][i]
        if st["pend"]:                      # own params not yet emitted
            for t in st["pend"]:
                t()
            st["pend"] = None
        last = i == len(st["rounds"]) - 1
        dead = last and x == "B"            # pt final col updates are dead
        if not dead:
            blocks = pair_emitters(r, st["m0"], st["mp"], st["mp"], st["co"])
            zip_emit(blocks, so["pend"] or [])
            so["pend"] = None
        st["idx"] = i + 1
        if not last:
            st["co"], st["pend"] = make_params(st["rounds"][i + 1],
                                               st["m0"], st["mp"])

    # 3:2 interleave: [A B A B A] x 7 covers 21 A-rounds and 14 B-rounds
    for step in range(7):
        for x in ("A", "A", "B", "A", "B"):
            do_slot(x)
    assert state["A"]["idx"] == len(A_ROUNDS)
    assert state["B"]["idx"] == len(B_ROUNDS)

    # ---- pt_a / pt_c diag min/max over i (final after B-stream) ----
    mn = main.tile([128, 2 * NTILES], f32, name="mn")[:]
    mx = main.tile([128, 2 * NTILES], f32, name="mx")[:]
    ptv = dv(NTILES, [[1, 2 * NTILES], [NM, 8]])
    nc.vector.tensor_reduce(mn, ptv, mybir.AxisListType.X, ALU.min)
    nc.vector.tensor_reduce(mx, ptv, mybir.AxisListType.X, ALU.max)

    # ---- tail (perturbative correction, sort, loss assembly) ----
    # Split into two independent m-halves so the two dependency chains
    # overlap; everything here is elementwise per (partition, m).
    acc_full = main.tile([128, NTILES], f32, tag="acc", name="acc")[:]
    HM = MRHO // 2

    def Q(tag, dt=f32):
        return main.tile([128, 8, 8, HM], dt, tag=tag, name=tag)[:]

    halves = []
    for hx, h0 in enumerate((0, HM)):
        SQ, S, dif, dif2 = Q(f"pSQ{hx}"), Q(f"pS{hx}"), Q(f"pdif{hx}"), Q(f"pdif2{hx}")
        corr = main.tile([128, 8, HM], f32, tag=f"pcorr{hx}", name="pcorr")[:]
        halves.append((h0, SQ, S, dif, dif2, corr))

    def emit_perturb(hx):
        h0, SQ, S, dif, dif2, corr = halves[hx]
        M = HM
        nc.scalar.activation(SQ, av(h0, [[SI, 8], [SJ, 8], [1, M]]), ACT.Square)
        nc.scalar.activation(S, av(SH + h0, [[SI, 8], [SJ, 8], [1, M]]),
                             ACT.Square)
        TT(S, S, SQ, ALU.add)
        TT(dif, dv(h0, [[NM, 8], [0, 8], [1, M]]),
           dv(h0, [[0, 8], [NM, 8], [1, M]]), ALU.subtract)    # d_i - d_j
        nc.scalar.activation(dif2, dif, ACT.Square)
        dif2_flat = bass.AP(tensor=dif2.tensor, offset=dif2.offset,
                            ap=[list(dif2.ap[0]), [1, 64 * M]])
        nc.vector.tensor_scalar(dif2_flat, dif2_flat, PDELTA, None, ALU.add)
        nc.vector.reciprocal(dif2, dif2)                       # R
        TT(SQ, S, dif, ALU.mult)                               # W = S*dif
        TT(SQ, SQ, dif2, ALU.mult)
        Wv = bass.AP(tensor=SQ.tensor, offset=SQ.offset,
                     ap=[list(SQ.ap[0]), [8 * M, 8], [1, M], [M, 8]])
        nc.vector.tensor_reduce(corr, Wv, mybir.AxisListType.X, ALU.add)
        TT(dv(h0, [[NM, 8], [1, M]]), dv(h0, [[NM, 8], [1, M]]), corr, ALU.add)

    def emit_sortassemble(hx):
        h0 = halves[hx][0]
        M = HM
        loc = {i: dv(i * NM + h0, [[1, M]]) for i in range(8)}
        tmin = main.tile([128, M], f32, tag=f"tmin{hx}", name="tmin")[:]
        ce = _CE8 if not (k0 == 4 and k1 == 4) else (_CE8[:16] + [(3, 4)])
        for (i, j) in ce:
            di, dj = loc[i], loc[j]
            TT(tmin, di, dj, ALU.min)
            TT(dj, di, dj, ALU.max)
            nc.vector.tensor_copy(di, tmin)

        def L(name):
            return main.tile([128, M], f32, tag=f"{name}{hx}", name=name)[:]

        mu_min = mn[:, h0:h0 + M]
        mu_max = mx[:, h0:h0 + M]
        nu_min = mn[:, NTILES + h0:NTILES + h0 + M]
        nu_max = mx[:, NTILES + h0:NTILES + h0 + M]
        acc = acc_full[:, h0:h0 + M]
        b0, b1 = L("b0"), L("b1")
        nc.vector.tensor_scalar(b0, loc[0], -8.0, 1.0, ALU.mult, ALU.add)
        nc.vector.reciprocal(b0, b0)
        nc.vector.tensor_scalar(b1, loc[7], -8.0, 1.0, ALU.mult, ALU.add)
        nc.vector.reciprocal(b1, b1)

        assert 1 <= k0 <= 8 and 1 <= k1 <= 8
        t1, t2_ = L("t1"), L("t2")
        sA, sB = L("sA"), L("sB")
        if k0 == 4:
            TT(sA, loc[0], loc[1], ALU.add)
            GT(sB, loc[2], loc[3], ALU.add)
            TT(t1, sA, sB, ALU.add)
        else:
            nc.gpsimd.tensor_copy(t1, loc[0])
            for i in range(1, k0):
                TT(t1, t1, loc[i], ALU.add)
        if k1 == 4:
            TT(sA, loc[7], loc[6], ALU.add)
            GT(sB, loc[5], loc[4], ALU.add)
            TT(t2_, sA, sB, ALU.add)
        else:
            nc.gpsimd.tensor_copy(t2_, loc[7])
            for i in range(6, 7 - k1, -1):
                TT(t2_, t2_, loc[i], ALU.add)
        u0, u1 = L("u0"), L("u1")
        STT(u0, t1, -k0 / 8.0, b0, ALU.add, ALU.mult)
        STT(u1, t2_, -k1 / 8.0, b1, ALU.add, ALU.mult)
        TT(u0, u0, u1, ALU.add)
        nc.vector.tensor_scalar(u0, u0, (k0 + k1) / 8.0, None, ALU.add)
        TT(acc, u0, u0, ALU.mult)
        t3s = [L(f"t3{i}") for i in range(4)]
        for n, (beta, ext) in enumerate(
                ((b0, mu_min), (b1, mu_max), (b0, nu_min), (b1, nu_max))):
            t3 = t3s[n]
            STT(t3, ext, -0.125, beta, ALU.add, ALU.mult)
            nc.vector.tensor_scalar(t3, t3, 0.125, None, ALU.add)
            if n % 2 == 0:
                TT(t3, t3, t3, ALU.mult)
            else:
                GT(t3, t3, t3, ALU.mult)
        TT(t3s[0], t3s[0], t3s[1], ALU.add)
        GT(t3s[2], t3s[2], t3s[3], ALU.add)
        TT(t3s[0], t3s[0], t3s[2], ALU.add)
        TT(acc, acc, t3s[0], ALU.add)

    emit_perturb(0)
    emit_perturb(1)
    emit_sortassemble(0)
    nc.sync.dma_start(out=out_d[:, 0:HM], in_=acc_full[:, 0:HM])
    emit_sortassemble(1)
    nc.sync.dma_start(out=out_d[:, HM:MRHO], in_=acc_full[:, HM:MRHO])


_prog_cache = {}


def kernel(rho_vec, rank0, rank1):
    rho_vec = np.asarray(rho_vec, dtype=np.float32)
    k0 = D - int(rank0)
    k1 = D - int(rank1)
    in_maps = _host_prep(rho_vec)

    from concourse.bass_utils import run_bass_kernel_spmd
    key = (k0, k1)
    if key not in _prog_cache:
        _prog_cache[key] = _build_program(k0, k1)
    nc = _prog_cache[key]
    res = run_bass_kernel_spmd(nc, in_maps, core_ids=list(range(NCORES)))
    return np.concatenate(
        [np.asarray(res.results[c]["out"]).T.reshape(-1) for c in range(NCORES)]
    ).astype(np.float32)


# revision 78
# speedup vs baseline: 1.0028x; 1.0004x over previous
"""Trainium2 Bass kernel for nn_BESNumEigen3qubitModel (v3).

Math reduction (exact): dm0/dm1 and their partial transposes are affine in
rho with the identity fixed, so every eigvalsh in the reference reduces to
eigenvalues of 3 Hermitian 8x8 matrices per batch element: rho, pt_a(rho),
pt_c(rho). With w = eig(rho) ascending, S_k0 = sum of k0 smallest, T_k1 =
sum of k1 largest, mu/nu = eig extrema of pt_a/pt_c:
   beta0 = 1/(1-8 w_min), beta1 = 1/(1-8 w_max)
   loss  = (beta0*(S_k0-k0/8)+k0/8 + beta1*(T_k1-k1/8)+k1/8)^2
           + sum over 4 PPT terms (beta*(ext-1/8)+1/8)^2.

Device algorithm (per core: 4096 batch elems -> 128 partitions x 32 tiles,
3 matrix types -> 96 matrices per partition):
  - Matrix data fp16, layout [128, h(2), i(8), j(8), m(96)] with the matrix
    index m LAST (stride 1) so every DVE operand is packed 2-byte ->
    2x (TensorTensor) / 4x (TensorCopy/TensorScalar) DVE throughput.
    The authoritative diagonal is kept in f32 [128, 8(i), 96(m)]
    (eigenvalues accumulate there at full precision).
  - Cyclic complex Jacobi in XOR-pair rounds (pairs (p, p^r), r = 1..7).
    Per round the 4 pairs' rotation params are computed batched over a
    [128, 4, m] layout (exact: a round's 2x2 pivot blocks are mutually
    disjoint); XOR-pair index sets are affine, so they are plain strided
    views. Then per-pair: 10-op fp16 column update, merged Hermitian row
    restore (rows <- conj(cols); the two racy entries are re-zeroed),
    fp16 diag mirror + annihilated-entry zeros.
  - Two independent round streams interleaved 3:2 with each stream's
    next-round params software-pipelined into the other stream's column
    updates: stream A = rho (21 rounds = 2 full + 1 extra sweep),
    stream B = pt_a/pt_c (14 rounds = 2 sweeps, final round params-only
    since only the f32 diag is read afterwards).
  - Round-1 pivot entries are DMA-prefetched into side tiles so the first
    params chain overlaps the bulk matrix DMA.
  - Tail: 2nd-order perturbative diag correction for rho from the residual
    off-diagonal (corr_i = sum_j |a_ij|^2 (d_i-d_j)/((d_i-d_j)^2+delta)),
    Batcher sort of the rho diag, min/max-reduce of PT diags, loss
    assembly - all split into two m-halves for chain overlap.
Accuracy (vs f64 reference, whole batch): max rel err ~5.3e-3.
"""

import numpy as np

D = 8
BATCH = 32768
NCORES = 8
PER_CORE = BATCH // NCORES       # 4096
NTILES = PER_CORE // 128         # 32 tiles per core
NM = 3 * NTILES                  # 96 matrices per partition (type-major)
MRHO = NTILES                    # rho-only count

# elem strides inside the fp16 matrix tile [2(h), 8(i), 8(j), NM(m)]
SM, SJ, SI, SH = 1, NM, 8 * NM, 64 * NM
ASIZE = 2 * 8 * 8 * NM           # 12288
PDELTA = 1e-6                    # perturbative-correction regularizer

_f32 = np.float32


# ---------------------------------------------------------------- host prep --

def _gellmann_basis(d):
    mats = []
    for j in range(d):
        for k in range(j + 1, d):
            m = np.zeros((d, d), np.complex128); m[j, k] = 1; m[k, j] = 1
            mats.append(m)
    for j in range(d):
        for k in range(j + 1, d):
            m = np.zeros((d, d), np.complex128); m[j, k] = -1j; m[k, j] = 1j
            mats.append(m)
    for l in range(1, d):
        m = np.zeros((d, d), np.complex128)
        m[np.arange(l), np.arange(l)] = 1
        m[l, l] = -l
        mats.append(np.sqrt(2.0 / (l * (l + 1))) * m)
    return np.stack(mats)


def _entry_perm(kind):
    p = np.zeros(64, np.int64)
    for i in range(8):
        for j in range(8):
            if kind == 'a':
                i2, j2 = (j & 4) | (i & 3), (i & 4) | (j & 3)
            else:
                i2, j2 = (i & 6) | (j & 1), (j & 6) | (i & 1)
            p[i * 8 + j] = i2 * 8 + j2
    return p


def _build_maps():
    """[64, 384] f32: (vec,1) -> 128 floats (f = h*64 + i*8 + j) of each of
    rho, pt_a(rho), pt_c(rho)."""
    G = _gellmann_basis(D)
    B = np.zeros((64, 128), np.float64)
    for k in range(63):
        B[k, :64] = G[k].real.reshape(-1)
        B[k, 64:] = G[k].imag.reshape(-1)
    B[63, :64] = (np.eye(D) / D).reshape(-1)

    def float_perm(kind):
        e = _entry_perm(kind)
        return np.concatenate([e, 64 + e])

    M3 = np.concatenate([B, B[:, float_perm('a')], B[:, float_perm('c')]], axis=1)
    return M3.astype(_f32)


_M3 = None


def _host_prep(rho_vec):
    global _M3
    if _M3 is None:
        _M3 = _build_maps()
    vec = rho_vec.astype(np.float64)
    vec = vec / np.linalg.norm(vec, axis=-1, keepdims=True)
    vec_aug = np.concatenate(
        [vec.astype(_f32), np.ones((vec.shape[0], 1), _f32)], axis=1)
    flat = vec_aug @ _M3                                   # [B, 384] f32
    arr = flat.reshape(NCORES, NTILES, 128, 3, 128)        # [core,tile,part,type,f]
    ins = []
    diag_f = np.array([i * 8 + i for i in range(8)])
    for c in range(NCORES):
        a = arr[c]
        # fp16 matrices: [part, f, type, tile] -> [128, f*96 + type*32 + tile]
        m16 = np.ascontiguousarray(
            a.transpose(1, 3, 2, 0).reshape(128, 128 * NM)).astype(np.float16)
        # f32 diag: [part, i, type, tile] -> [128, i*96 + m]
        dg = np.ascontiguousarray(
            a[:, :, :, diag_f].transpose(1, 3, 2, 0).reshape(128, 8 * NM)
        ).astype(_f32)
        ins.append({"mats": m16, "diag": dg})
    return ins


# ------------------------------------------------------------ device kernel --

def _xor_pairs(r):
    return [(i, i ^ r) for i in range(8) if i < (i ^ r)]


def _enum_bits(r):
    """Enumeration bit-steps (descending) for pset = {p: bit_bmax(r)(p)=0},
    enumerated in ascending-p order."""
    bmax = 4 if r >= 4 else (2 if r >= 2 else 1)
    return [b for b in (4, 2, 1) if b != bmax]


# Batcher odd-even mergesort network for 8 elements (19 comparators)
_CE8 = [(0, 1), (2, 3), (4, 5), (6, 7), (0, 2), (1, 3), (4, 6), (5, 7),
        (1, 2), (5, 6), (0, 4), (1, 5), (2, 6), (3, 7), (2, 4), (3, 5),
        (1, 2), (3, 4), (5, 6)]

N_FULL = 2      # full sweeps (all 3 matrix types)
N_RHO = 1       # extra rho-only sweeps


def _build_program(k0, k1):
    import concourse.bass as bass
    import concourse.bacc as bacc
    import concourse.mybir as mybir
    from concourse.tile import TileContext
    from contextlib import ExitStack

    f32 = mybir.dt.float32
    f16 = mybir.dt.float16
    ALU = mybir.AluOpType
    ACT = mybir.ActivationFunctionType

    nc = bacc.Bacc("TRN2")
    mats_d = nc.dram_tensor("mats", [128, ASIZE], f16, kind="ExternalInput")
    diag_d = nc.dram_tensor("diag", [128, 8 * NM], f32, kind="ExternalInput")
    out_d = nc.dram_tensor("out", [128, NTILES], f32, kind="ExternalOutput")

    with ExitStack() as ctx:
        tc = ctx.enter_context(TileContext(nc))
        main = ctx.enter_context(tc.tile_pool(name="main", bufs=1))
        pp = ctx.enter_context(tc.tile_pool(name="pp", bufs=2))
        cp = ctx.enter_context(tc.tile_pool(name="cp", bufs=2))

        A = main.tile([128, ASIZE], f16, name="A")
        Dg = main.tile([128, 8 * NM], f32, name="Dg")
        Aap = A[:]
        Dap = Dg[:]
        pdim = list(Aap.ap[0])

        def av(offset, dims):
            return bass.AP(tensor=Aap.tensor, offset=Aap.offset + offset,
                           ap=[pdim] + dims)

        def dv(offset, dims):
            return bass.AP(tensor=Dap.tensor, offset=Dap.offset + offset,
                           ap=[list(Dap.ap[0])] + dims)

        nc.sync.dma_start(out=Dg[:], in_=diag_d[:, :])
        # Prefetch round-1 pivot entries (f = p*8+q and the im half) on the
        # Act engine's DMA queue so the first params chain fully overlaps the
        # bulk matrix DMA below. r=1 pairs (0,1),(2,3),(4,5),(6,7) -> f = 1,
        # 19, 37, 55 (re) and +64 (im), each an 18-step affine f-row set.
        xpre = main.tile([128, 4 * NM], f16, name="xpre")
        ypre = main.tile([128, 4 * NM], f16, name="ypre")
        md = mats_d[:, :]

        def mrows(foff):
            return bass.AP(tensor=md.tensor, offset=md.offset + foff * NM,
                           ap=[list(md.ap[0]), [18 * NM, 4], [1, NM]])

        nc.sync.dma_start(out=xpre[:], in_=mrows(1))
        nc.sync.dma_start(out=ypre[:], in_=mrows(65))
        NCHUNK = 2
        for ch in range(NCHUNK):
            w = ASIZE // NCHUNK
            nc.sync.dma_start(out=av(ch * w, [[1, w]]),
                              in_=mats_d[:, ch * w:(ch + 1) * w])

        eps30 = main.tile([128, 1], f32, name="eps30")
        nc.vector.memset(eps30[:], 1e-30)
        eps35 = main.tile([128, 1], f32, name="eps35")
        nc.vector.memset(eps35[:], 1e-35)

        with nc.allow_low_precision(reason="fp16 Jacobi data by design"):
            _emit_jacobi(nc, bass, mybir, main, pp, cp, av, dv,
                         eps30, eps35, out_d, k0, k1, xpre, ypre)

    nc.finalize()
    return nc


def _emit_jacobi(nc, bass, mybir, main, pp, cp, av, dv, eps30, eps35,
                 out_d, k0, k1, xpre, ypre):
    f32 = mybir.dt.float32
    f16 = mybir.dt.float16
    ALU = mybir.AluOpType
    ACT = mybir.ActivationFunctionType
    TT = nc.vector.tensor_tensor
    GT = nc.gpsimd.tensor_tensor
    STT = nc.vector.scalar_tensor_tensor

    def make_params(r, m0, mp, xy_override=None):
        """Rotation params for round r, matrices m in [m0, m0+mp).
        Returns ((c16, sr16, s2c), [op thunks]) - thunks emit one op each,
        in dependency order, so callers can interleave them with other work.
        xy_override: (Xv, Yv) APs to read the pivot entries from instead of
        the matrix tile (used for the DMA-prefetched first round)."""
        b1, b2 = _enum_bits(r)  # descending

        def merged(dims):
            if dims[0][0] == 2 * dims[1][0]:
                return [[dims[1][0], 4]] + dims[2:]
            return dims

        sgn = lambda b: -1 if (r & b) else 1
        xdims = merged([[b1 * SI + sgn(b1) * b1 * SJ, 2],
                        [b2 * SI + sgn(b2) * b2 * SJ, 2], [1, mp]])
        if xy_override is not None:
            Xv, Yv = xy_override
        else:
            Xv = av(r * SJ + m0, list(xdims))
            Yv = av(SH + r * SJ + m0, list(xdims))
        appv = dv(m0, merged([[b1 * NM, 2], [b2 * NM, 2], [1, mp]]))
        aqqv = dv(r * NM + m0, merged([[sgn(b1) * b1 * NM, 2],
                                       [sgn(b2) * b2 * NM, 2], [1, mp]]))

        def P(tag, dt=f32):
            return pp.tile([128, 4, mp], dt, tag=f"{tag}{mp}g{m0}", name=tag)[:]

        sqx, sqy, n2p, g = P("sqx"), P("sqy"), P("n2p"), P("g")
        gsq, s2, h, ag = P("gsq"), P("s2"), P("h"), P("ag")
        den, T, sg, T2 = P("den"), P("T"), P("sg"), P("T2")
        t2, cden, u, urb2 = P("t2"), P("cden"), P("u"), P("urb2")
        tb = P("tb")
        c16 = P("c16", f16)
        sr16 = P("sr16", f16)
        s2c = pp.tile([128, 2, 4, mp], f16, tag=f"s2c{mp}g{m0}", name="s2c")[:]

        ops = [
            lambda: nc.scalar.activation(sqx, Xv, ACT.Square, scale=2.0),
            lambda: nc.scalar.activation(sqy, Yv, ACT.Square, scale=2.0),
            lambda: TT(g, appv, aqqv, ALU.subtract),
            lambda: nc.scalar.activation(ag, g, ACT.Abs),
            lambda: nc.scalar.sign(sg, g, bias=eps35[:]),
            lambda: nc.scalar.activation(gsq, g, ACT.Square),
            lambda: TT(n2p, sqx, sqy, ALU.add),
            lambda: TT(s2, gsq, n2p, ALU.add),
            lambda: nc.scalar.activation(h, s2, ACT.Sqrt, bias=eps30[:]),
            lambda: GT(den, ag, h, ALU.add),
            lambda: nc.vector.reciprocal(T, den),
            lambda: GT(T2, T, T, ALU.mult),
            lambda: TT(t2, n2p, T2, ALU.mult),
            lambda: nc.scalar.activation(cden, t2, ACT.Sqrt, bias=1.0),
            lambda: nc.vector.reciprocal(c16, cden),
            lambda: GT(u, T, sg, ALU.mult),
            lambda: STT(urb2, u, 2.0, c16, ALU.mult, ALU.mult),
            lambda: TT(sr16, urb2, Xv, ALU.mult),
            lambda: TT(s2c[:, 0], urb2, Yv, ALU.mult),
            lambda: nc.scalar.activation(s2c[:, 1], s2c[:, 0], ACT.Copy,
                                         scale=-1.0),
            lambda: STT(tb, u, 0.5, n2p, ALU.mult, ALU.mult),
            lambda: GT(appv, appv, tb, ALU.add),
            lambda: GT(aqqv, aqqv, tb, ALU.subtract),
        ]
        return (c16, sr16, s2c), ops

    def emit_pair(k, p, q, m0, mu, mp, coeffs, cofs=0):
        """Column update + Hermitian restore for pair (p, q), m in [m0, m0+mu).
        cofs: m-offset of this update range inside the coefficient tiles."""
        c16, sr16, s2c = coeffs
        UD = [[SH, 2], [SI, 8], [1, mu]]
        UDsw = [[-SH, 2], [SI, 8], [1, mu]]
        colp = av(p * SJ + m0, list(UD))
        colq = av(q * SJ + m0, list(UD))
        colp_sw = av(SH + p * SJ + m0, list(UDsw))
        colq_sw = av(SH + q * SJ + m0, list(UDsw))
        cb = bass.AP(tensor=c16.tensor, offset=c16.offset + k * mp + cofs,
                     ap=[list(c16.ap[0]), [0, 2], [0, 8], [1, mu]])
        srb = bass.AP(tensor=sr16.tensor, offset=sr16.offset + k * mp + cofs,
                      ap=[list(sr16.ap[0]), [0, 2], [0, 8], [1, mu]])
        s2b = bass.AP(tensor=s2c.tensor, offset=s2c.offset + k * mp + cofs,
                      ap=[list(s2c.ap[0]), [4 * mp, 2], [0, 8], [1, mu]])

        def CW(tag):
            return cp.tile([128, 2, 8, mu], f16, tag=f"{tag}{mu}g{m0}",
                           name=tag)[:]

        tP, uP, tQ, uQ = CW("tP"), CW("uP"), CW("tQ"), CW("uQ")
        TT(tP, srb, colq, ALU.mult)
        TT(uP, s2b, colq_sw, ALU.mult)
        GT(tQ, srb, colp, ALU.mult)
        TT(uQ, s2b, colp_sw, ALU.mult)
        TT(colp, cb, colp, ALU.mult)
        TT(colp, colp, tP, ALU.add)
        TT(colp, colp, uP, ALU.add)
        TT(colq, cb, colq, ALU.mult)
        TT(colq, colq, tQ, ALU.subtract)
        TT(colq, colq, uQ, ALU.add)

        # Hermitian row restore (merged rows p,q): rows <- conj(cols).
        # The (p,q)/(q,p) entries race within the merged ops but are
        # explicitly zeroed below.
        dROW = [[(q - p) * SI, 2], [SJ, 8], [1, mu]]
        sCOL = [[(q - p) * SJ, 2], [SI, 8], [1, mu]]
        nc.scalar.copy(av(p * SI + m0, list(dROW)), av(p * SJ + m0, list(sCOL)))
        for rw in (p, q):
            nc.vector.tensor_scalar(
                av(SH + rw * SI + m0, [[SJ, 8], [1, mu]]),
                av(SH + rw * SJ + m0, [[SI, 8], [1, mu]]),
                -1.0, None, ALU.mult)

        # diag mirror (fp16 <- f32 Dg) + annihilated-entry zeros
        mdst = av(p * (SI + SJ) + m0, [[(q - p) * (SI + SJ), 2], [1, mu]])
        msrc = dv(p * NM + m0, [[(q - p) * NM, 2], [1, mu]])
        nc.gpsimd.tensor_copy(mdst, msrc)
        nc.scalar.memzero(av(SH + p * (SI + SJ) + m0,
                             [[(q - p) * SI, 2], [(q - p) * SJ, 2], [1, mu]]))
        nc.gpsimd.memset(av(p * SI + q * SJ + m0,
                            [[(q - p) * (SI - SJ), 2], [1, mu]]), 0.0)

    # ---- sweeps: two type-aligned streams, interleaved 3:2 ----
    # Stream A: rho (m 0..31), N_FULL + N_RHO sweeps = 21 rounds (its extra
    # rho-only sweeps overlap the PT stream instead of trailing serially).
    # Stream B: pt_a/pt_c (m 32..95), N_FULL sweeps = 14 rounds; the last
    # round's column updates are dead (only the f32 diag is read afterwards)
    # and are skipped.
    # Each stream's next-round params are software-pipelined into the other
    # stream's pair updates (zip_emit) to hide the params dependency chain.

    def pair_emitters(r, m0, mu, mp, co, cofs=0):
        return [(lambda k=k, p=p, q=q: emit_pair(k, p, q, m0, mu, mp, co, cofs))
                for k, (p, q) in enumerate(_xor_pairs(r))]

    def zip_emit(blocks, thunks):
        per = max(1, len(thunks) // len(blocks) - 1) if thunks else 0
        for i, b in enumerate(blocks):
            b()
            hi = (i + 1) * per if i < len(blocks) - 1 else len(thunks)
            for t in thunks[i * per:hi]:
                t()

    A_ROUNDS = [r for s in range(N_FULL + N_RHO) for r in range(1, 8)]
    B_ROUNDS = [r for s in range(N_FULL) for r in range(1, 8)]
    AM0, AMP = 0, MRHO          # rho range
    BM0, BMP = MRHO, NM - MRHO  # pt range

    state = {
        "A": {"rounds": A_ROUNDS, "idx": 0, "m0": AM0, "mp": AMP,
              "co": None, "pend": None},
        "B": {"rounds": B_ROUNDS, "idx": 0, "m0": BM0, "mp": BMP,
              "co": None, "pend": None},
    }
    for nm in ("A", "B"):
        st = state[nm]
        xo = bass.AP(tensor=xpre[:].tensor,
                     offset=xpre[:].offset + st["m0"],
                     ap=[list(xpre[:].ap[0]), [NM, 4], [1, st["mp"]]])
        yo = bass.AP(tensor=ypre[:].tensor,
                     offset=ypre[:].offset + st["m0"],
                     ap=[list(ypre[:].ap[0]), [NM, 4], [1, st["mp"]]])
        st["co"], st["pend"] = make_params(st["rounds"][0], st["m0"], st["mp"],
                                           xy_override=(xo, yo))

    def do_slot(x):
        other = "B" if x == "A" else "A"
        st, so = state[x], state[other]
        i = st["idx"]
        r = st["rounds"][i]
        if st["pend"]:                      # own params not yet emitted
            for t in st["pend"]:
                t()
            st["pend"] = None
        last = i == len(st["rounds"]) - 1
        dead = last and x == "B"            # pt final col updates are dead
        if not dead:
            blocks = pair_emitters(r, st["m0"], st["mp"], st["mp"], st["co"])
            zip_emit(blocks, so["pend"] or [])
            so["pend"] = None
        st["idx"] = i + 1
        if not last:
            st["co"], st["pend"] = make_params(st["rounds"][i + 1],
                                               st["m0"], st["mp"])

    # 3:2 interleave: [A B A B A] x 7 covers 21 A-rounds and 14 B-rounds
    for step in range(7):
        for x in ("A", "A", "B", "A", "B"):
            do_slot(x)
    assert state["A"]["idx"] == len(A_ROUNDS)
    assert state["B"]["idx"] == len(B_ROUNDS)

    # ---- pt_a / pt_c diag min/max over i (final after B-stream) ----
    mn = main.tile([128, 2 * NTILES], f32, name="mn")[:]
    mx = main.tile([128, 2 * NTILES], f32, name="mx")[:]
    ptv = dv(NTILES, [[1, 2 * NTILES], [NM, 8]])
    nc.vector.tensor_reduce(mn, ptv, mybir.AxisListType.X, ALU.min)
    nc.vector.tensor_reduce(mx, ptv, mybir.AxisListType.X, ALU.max)

    # ---- tail (perturbative correction, sort, loss assembly) ----
    # Split into two independent m-halves so the two dependency chains
    # overlap; everything here is elementwise per (partition, m).
    acc_full = main.tile([128, NTILES], f32, tag="acc", name="acc")[:]
    HM = MRHO // 2

    def Q(tag, dt=f32):
        return main.tile([128, 8, 8, HM], dt, tag=tag, name=tag)[:]

    halves = []
    for hx, h0 in enumerate((0, HM)):
        SQ, S, dif, dif2 = Q(f"pSQ{hx}"), Q(f"pS{hx}"), Q(f"pdif{hx}"), Q(f"pdif2{hx}")
        corr = main.tile([128, 8, HM], f32, tag=f"pcorr{hx}", name="pcorr")[:]
        halves.append((h0, SQ, S, dif, dif2, corr))

    def emit_perturb(hx):
        h0, SQ, S, dif, dif2, corr = halves[hx]
        M = HM
        nc.scalar.activation(SQ, av(h0, [[SI, 8], [SJ, 8], [1, M]]), ACT.Square)
        nc.scalar.activation(S, av(SH + h0, [[SI, 8], [SJ, 8], [1, M]]),
                             ACT.Square)
        TT(S, S, SQ, ALU.add)
        TT(dif, dv(h0, [[NM, 8], [0, 8], [1, M]]),
           dv(h0, [[0, 8], [NM, 8], [1, M]]), ALU.subtract)    # d_i - d_j
        nc.scalar.activation(dif2, dif, ACT.Square)
        dif2_flat = bass.AP(tensor=dif2.tensor, offset=dif2.offset,
                            ap=[list(dif2.ap[0]), [1, 64 * M]])
        nc.vector.tensor_scalar(dif2_flat, dif2_flat, PDELTA, None, ALU.add)
        nc.vector.reciprocal(dif2, dif2)                       # R
        TT(SQ, S, dif, ALU.mult)                               # W = S*dif
        TT(SQ, SQ, dif2, ALU.mult)
        Wv = bass.AP(tensor=SQ.tensor, offset=SQ.offset,
                     ap=[list(SQ.ap[0]), [8 * M, 8], [1, M], [M, 8]])
        nc.vector.tensor_reduce(corr, Wv, mybir.AxisListType.X, ALU.add)
        TT(dv(h0, [[NM, 8], [1, M]]), dv(h0, [[NM, 8], [1, M]]), corr, ALU.add)

    def emit_sortassemble(hx):
        h0 = halves[hx][0]
        M = HM
        loc = {i: dv(i * NM + h0, [[1, M]]) for i in range(8)}
        tmin = main.tile([128, M], f32, tag=f"tmin{hx}", name="tmin")[:]
        ce = _CE8 if not (k0 == 4 and k1 == 4) else (_CE8[:16] + [(3, 4)])
        for (i, j) in ce:
            di, dj = loc[i], loc[j]
            TT(tmin, di, dj, ALU.min)
            TT(dj, di, dj, ALU.max)
            nc.vector.tensor_copy(di, tmin)

        def L(name):
            return main.tile([128, M], f32, tag=f"{name}{hx}", name=name)[:]

        mu_min = mn[:, h0:h0 + M]
        mu_max = mx[:, h0:h0 + M]
        nu_min = mn[:, NTILES + h0:NTILES + h0 + M]
        nu_max = mx[:, NTILES + h0:NTILES + h0 + M]
        acc = acc_full[:, h0:h0 + M]
        b0, b1 = L("b0"), L("b1")
        nc.vector.tensor_scalar(b0, loc[0], -8.0, 1.0, ALU.mult, ALU.add)
        nc.vector.reciprocal(b0, b0)
        nc.vector.tensor_scalar(b1, loc[7], -8.0, 1.0, ALU.mult, ALU.add)
        nc.vector.reciprocal(b1, b1)

        assert 1 <= k0 <= 8 and 1 <= k1 <= 8
        t1, t2_ = L("t1"), L("t2")
        sA, sB = L("sA"), L("sB")
        if k0 == 4:
            TT(sA, loc[0], loc[1], ALU.add)
            GT(sB, loc[2], loc[3], ALU.add)
            TT(t1, sA, sB, ALU.add)
        else:
            nc.gpsimd.tensor_copy(t1, loc[0])
            for i in range(1, k0):
                TT(t1, t1, loc[i], ALU.add)
        if k1 == 4:
            TT(sA, loc[7], loc[6], ALU.add)
            GT(sB, loc[5], loc[4], ALU.add)
            TT(t2_, sA, sB, ALU.add)
        else:
            nc.gpsimd.tensor_copy(t2_, loc[7])
            for i in range(6, 7 - k1, -1):
                TT(t2_, t2_, loc[i], ALU.add)
        u0, u1 = L("u0"), L("u1")
        STT(u0, t1, -k0 / 8.0, b0, ALU.add, ALU.mult)
        STT(u1, t2_, -k1 / 8.0, b1, ALU.add, ALU.mult)
        TT(u0, u0, u1, ALU.add)
        nc.vector.tensor_scalar(u0, u0, (k0 + k1) / 8.0, None, ALU.add)
        TT(acc, u0, u0, ALU.mult)
        t3s = [L(f"t3{i}") for i in range(4)]
        for n, (beta, ext) in enumerate(
                ((b0, mu_min), (b1, mu_max), (b0, nu_min), (b1, nu_max))):
            t3 = t3s[n]
            STT(t3, ext, -0.125, beta, ALU.add, ALU.mult)
            nc.vector.tensor_scalar(t3, t3, 0.125, None, ALU.add)
            if n % 2 == 0:
                TT(t3, t3, t3, ALU.mult)
            else:
                GT(t3, t3, t3, ALU.mult)
        TT(t3s[0], t3s[0], t3s[1], ALU.add)
        GT(t3s[2], t3s[2], t3s[3], ALU.add)
        TT(t3s[0], t3s[0], t3s[2], ALU.add)
        TT(acc, acc, t3s[0], ALU.add)

    emit_perturb(0)
    emit_perturb(1)
    emit_sortassemble(0)
    nc.sync.dma_start(out=out_d[:, 0:HM], in_=acc_full[:, 0:HM])
    emit_sortassemble(1)
    nc.sync.dma_start(out=out_d[:, HM:MRHO], in_=acc_full[:, HM:MRHO])


_prog_cache = {}


def kernel(rho_vec, rank0, rank1):
    rho_vec = np.asarray(rho_vec, dtype=np.float32)
    k0 = D - int(rank0)
    k1 = D - int(rank1)
    in_maps = _host_prep(rho_vec)

    from concourse.bass_utils import run_bass_kernel_spmd
    key = (k0, k1)
    if key not in _prog_cache:
        _prog_cache[key] = _build_program(k0, k1)
    nc = _prog_cache[key]
    res = run_bass_kernel_spmd(nc, in_maps, core_ids=list(range(NCORES)))
    return np.concatenate(
        [np.asarray(res.results[c]["out"]).T.reshape(-1) for c in range(NCORES)]
    ).astype(np.float32)
